# revision 2
# baseline (speedup 1.0000x reference)
"""Trainium2 Bass kernel for the LogicMessagePassingNetwork problem.

Reference computation (E=1M edges, T=2M triangles, R=50, D=64):
    x   = edge_feat + relation_emb[edge_rel]                      # [E, D]
    m   = relu((x[edge_ab] * x[edge_bc]) @ W_msg)                 # [T, D]
    agg = segment_sum(m, edge_ac, E)                              # [E, D]
    out = relu(x + agg @ W_upd)                                   # [E, D]

Strategy (8 cores, no collectives):
  - Host folds relation_emb into the table: xtbl = edge_feat + rel_emb[rel]
    (one [E,64] f32 table; gathers on device fetch one row per triangle
    side instead of two).
  - Host sorts triangles by edge_ac; core k owns output edges
    [k*E/8, (k+1)*E/8) and exactly the triangles whose ac falls there.
  - Per core, output edges are processed in blocks of BLK=512. The
    block's triangles are padded to TB chunks of 128 slots (TB is the
    global max, ~9, so the program is identical across cores / SPMD).
  - Chunks are processed in dual-groups of 2: ONE indirect DMA gathers
    512 rows (xa,xb for both chunks) into a [128,256] tile; prod=xa*xb
    (DVE), PE-transpose, m=relu(prodT^T @ W_msg), one-hot [128,512]
    against the block's 512 edge slots (fp16), scatter-matmul
    accumulates aggT[64,512] f32 in PSUM.
  - Block epilogue: upd = aggT^T @ W_upd (4 matmuls), out =
    relu(own_x + upd) with own_x streamed as one contiguous 128KB DMA
    (host pre-permuted), written back as one 128KB DMA.
"""
import numpy as np

E = 1_000_000
T = 2_000_000
R = 50
D = 64
NCORES = 8
EPC = E // NCORES          # edges per core (125000)
BLK = 512                  # output edges per block
NBLK = (EPC + BLK - 1) // BLK          # 245 blocks/core
EPAD = NBLK * BLK                      # padded edges/core (125440)
TPAD = 512                             # extra zero rows at table end
ARPAD = 60000.0                        # fp16-exact, never matches iota


# ----------------------------------------------------------------- host prep
def host_preprocess(edge_rel, edge_ab, edge_bc, edge_ac):
    """Index-space preprocessing. Returns per-core index arrays + TB."""
    ab = np.asarray(edge_ab).astype(np.int64)
    bc = np.asarray(edge_bc).astype(np.int64)
    ac = np.asarray(edge_ac).astype(np.int64)

    order = np.argsort(ac, kind="stable")
    ab_s, bc_s, ac_s = ab[order], bc[order], ac[order]

    cores = []
    TB = 0
    for k in range(NCORES):
        lo, hi = np.searchsorted(ac_s, [k * EPC, (k + 1) * EPC])
        c_ab = ab_s[lo:hi].astype(np.int32)
        c_bc = bc_s[lo:hi].astype(np.int32)
        ac_l = ac_s[lo:hi] - k * EPC
        blk = ac_l // BLK
        cnt = np.bincount(blk, minlength=NBLK)
        TB = max(TB, -(-int(cnt.max()) // 128))
        cores.append((c_ab, c_bc, ac_l, blk, cnt))

    ND, NS = TB // 2, TB % 2
    outs = []
    for k in range(NCORES):
        c_ab, c_bc, ac_l, blk, cnt = cores[k]
        starts = np.zeros(NBLK, np.int64)
        starts[1:] = np.cumsum(cnt)[:-1]
        rank = np.arange(len(ac_l)) - starts[blk]
        chunk = rank // 128
        p = rank % 128

        gab = np.zeros((NBLK, TB, 128), np.int32)
        gbc = np.zeros((NBLK, TB, 128), np.int32)
        gar = np.full((NBLK, TB, 128), ARPAD, np.float16)
        gab[blk, chunk, p] = c_ab
        gbc[blk, chunk, p] = c_bc
        gar[blk, chunk, p] = (ac_l % BLK).astype(np.float16)

        idx = np.zeros((NBLK, 128, 2 * TB), np.int32)
        for g in range(ND):
            idx[:, :, 4 * g + 0] = gab[:, 2 * g, :]
            idx[:, :, 4 * g + 1] = gab[:, 2 * g + 1, :]
            idx[:, :, 4 * g + 2] = gbc[:, 2 * g, :]
            idx[:, :, 4 * g + 3] = gbc[:, 2 * g + 1, :]
        if NS:
            idx[:, :, 4 * ND + 0] = gab[:, TB - 1, :]
            idx[:, :, 4 * ND + 1] = gbc[:, TB - 1, :]
        arh = np.ascontiguousarray(gar.transpose(0, 2, 1))  # [NBLK,128,TB]
        outs.append(dict(idx=idx, arh=arh))
    return outs, TB


def build_table(edge_feat, relation_emb, edge_rel):
    """xtbl[i] = edge_feat[i] + relation_emb[edge_rel[i]], padded rows zero."""
    rel = np.asarray(edge_rel).astype(np.int64)
    tbl = np.zeros((E + TPAD, D), np.float32)
    tbl[:E] = np.asarray(edge_feat, np.float32) + \
        np.asarray(relation_emb, np.float32)[rel]
    return tbl


def build_own(tbl, k):
    """Pre-permuted own-edge x rows: own[b, p, j*64:(j+1)*64] = x[lo+b*512+j*128+p]."""
    ox = tbl[k * EPC: k * EPC + EPAD]                      # [125440, 64]
    return np.ascontiguousarray(
        ox.reshape(NBLK, 4, 128, D).transpose(0, 2, 1, 3).reshape(NBLK, 128, 4 * D))


def unpermute_out(o):
    """[NBLK,128,256] -> [EPC,64]"""
    return o.reshape(NBLK, 128, 4, D).transpose(0, 2, 1, 3).reshape(EPAD, D)[:EPC]


# ------------------------------------------------------------- device kernel
def build_bass(TB, nblk):
    import concourse.bass as bass
    import concourse.bacc as bacc
    import concourse.mybir as mybir
    import concourse.tile as tile
    from concourse.masks import make_identity

    f32 = mybir.dt.float32
    f16 = mybir.dt.float16
    i32 = mybir.dt.int32
    ND, NS = TB // 2, TB % 2
    nc = bacc.Bacc(None, target_bir_lowering=False)

    tbl = nc.dram_tensor("tbl", [E + TPAD, D], f32, kind="ExternalInput")
    own = nc.dram_tensor("own", [nblk, 128, 4 * D], f32, kind="ExternalInput")
    idx = nc.dram_tensor("idx", [nblk, 128, 2 * TB], i32, kind="ExternalInput")
    arh = nc.dram_tensor("arh", [nblk, 128, TB], f16, kind="ExternalInput")
    iota = nc.dram_tensor("iota", [128, BLK], f16, kind="ExternalInput")
    wmsg = nc.dram_tensor("wmsg", [D, D], f16, kind="ExternalInput")
    wupd = nc.dram_tensor("wupd", [D, D], f16, kind="ExternalInput")
    out = nc.dram_tensor("out", [nblk, 128, 4 * D], f32, kind="ExternalOutput")

    with tile.TileContext(nc) as tc:
        with tc.tile_pool(name="const", bufs=1) as cpool, \
             tc.tile_pool(name="gath", bufs=6) as gpool, \
             tc.tile_pool(name="idxp", bufs=3) as ipool, \
             tc.tile_pool(name="work", bufs=4) as wpool, \
             tc.tile_pool(name="ohp", bufs=4) as ohpool, \
             tc.tile_pool(name="outp", bufs=3) as opool, \
             tc.tile_pool(name="ps", bufs=2, space="PSUM") as pspool, \
             tc.tile_pool(name="psagg", bufs=2, space="PSUM") as paggpool, \
             tc.tile_pool(name="psupd", bufs=2, space="PSUM") as pupdpool:

            wmsg_sb = cpool.tile([D, D], f16)
            nc.sync.dma_start(out=wmsg_sb[:], in_=wmsg[:])
            wupd_sb = cpool.tile([D, D], f16)
            nc.sync.dma_start(out=wupd_sb[:], in_=wupd[:])
            iota_sb = cpool.tile([128, BLK], f16)
            nc.sync.dma_start(out=iota_sb[:], in_=iota[:])
            ident = cpool.tile([128, 128], f32)
            make_identity(nc, ident[:])

            for b in range(nblk):
                idx_t = ipool.tile([128, 2 * TB], i32, tag="idx")
                nc.sync.dma_start(out=idx_t[:], in_=idx[b])
                arh_t = ipool.tile([128, TB], f16, tag="arh")
                nc.sync.dma_start(out=arh_t[:], in_=arh[b])

                aggT = paggpool.tile([D, BLK], f32, space="PSUM", tag="aggT")

                def do_chunks(lhsT_m2, oh2, nch, first, last):
                    # scatter-accumulate nch chunks into aggT
                    for c in range(nch):
                        nc.tensor.matmul(
                            out=aggT[:],
                            lhsT=lhsT_m2[:, c * D:(c + 1) * D],
                            rhs=oh2[:, c * BLK:(c + 1) * BLK],
                            start=(first and c == 0), stop=(last and c == nch - 1))

                for g in range(ND):
                    gt = gpool.tile([128, 4 * D], f32, tag="g2")
                    nc.gpsimd.indirect_dma_start(
                        out=gt[:], out_offset=None, in_=tbl[:],
                        in_offset=bass.IndirectOffsetOnAxis(
                            ap=idx_t[:, 4 * g:4 * g + 4], axis=0))
                    prod2 = wpool.tile([128, 128], f32, tag="prod2")
                    nc.vector.tensor_mul(out=prod2[:], in0=gt[:, 0:128],
                                         in1=gt[:, 128:256])
                    prodT_ps = pspool.tile([128, 128], f32, space="PSUM", tag="prodT")
                    nc.tensor.transpose(out=prodT_ps[:], in_=prod2[:], identity=ident[:])
                    prodT = wpool.tile([128, 128], f16, tag="prodTs")
                    nc.scalar.activation(out=prodT[:], in_=prodT_ps[:],
                                         func=mybir.ActivationFunctionType.Copy)
                    m_ps = pspool.tile([128, 128], f32, space="PSUM", tag="mps")
                    nc.tensor.matmul(out=m_ps[:, 0:D], lhsT=prodT[0:D, :],
                                     rhs=wmsg_sb[:], start=True, stop=True)
                    nc.tensor.matmul(out=m_ps[:, D:2 * D], lhsT=prodT[D:128, :],
                                     rhs=wmsg_sb[:], start=True, stop=True)
                    m2 = wpool.tile([128, 128], f16, tag="m2")
                    nc.scalar.activation(out=m2[:], in_=m_ps[:],
                                         func=mybir.ActivationFunctionType.Relu)
                    oh2 = ohpool.tile([128, 2 * BLK], f16, tag="oh2")
                    nc.vector.tensor_tensor(
                        out=oh2[:, 0:BLK],
                        in0=arh_t[:, 2 * g:2 * g + 1].to_broadcast([128, BLK]),
                        in1=iota_sb[:], op=mybir.AluOpType.is_equal)
                    nc.vector.tensor_tensor(
                        out=oh2[:, BLK:2 * BLK],
                        in0=arh_t[:, 2 * g + 1:2 * g + 2].to_broadcast([128, BLK]),
                        in1=iota_sb[:], op=mybir.AluOpType.is_equal)
                    do_chunks(m2, oh2, 2, first=(g == 0), last=(NS == 0 and g == ND - 1))

                if NS:
                    gt = gpool.tile([128, 2 * D], f32, tag="g1")
                    nc.gpsimd.indirect_dma_start(
                        out=gt[:], out_offset=None, in_=tbl[:],
                        in_offset=bass.IndirectOffsetOnAxis(
                            ap=idx_t[:, 4 * ND:4 * ND + 2], axis=0))
                    prod1 = wpool.tile([128, D], f32, tag="prod1")
                    nc.vector.tensor_mul(out=prod1[:], in0=gt[:, 0:D],
                                         in1=gt[:, D:2 * D])
                    prodT_ps = pspool.tile([128, 128], f32, space="PSUM", tag="prodT")
                    nc.tensor.transpose(out=prodT_ps[0:D, :], in_=prod1[:],
                                        identity=ident[:])
                    prodT = wpool.tile([128, 128], f16, tag="prodTs")
                    nc.scalar.activation(out=prodT[0:D, :], in_=prodT_ps[0:D, :],
                                         func=mybir.ActivationFunctionType.Copy)
                    m_ps = pspool.tile([128, 128], f32, space="PSUM", tag="mps")
                    nc.tensor.matmul(out=m_ps[:, 0:D], lhsT=prodT[0:D, :],
                                     rhs=wmsg_sb[:], start=True, stop=True)
                    m2 = wpool.tile([128, 128], f16, tag="m2")
                    nc.scalar.activation(out=m2[:, 0:D], in_=m_ps[:, 0:D],
                                         func=mybir.ActivationFunctionType.Relu)
                    oh2 = ohpool.tile([128, 2 * BLK], f16, tag="oh2")
                    nc.vector.tensor_tensor(
                        out=oh2[:, 0:BLK],
                        in0=arh_t[:, TB - 1:TB].to_broadcast([128, BLK]),
                        in1=iota_sb[:], op=mybir.AluOpType.is_equal)
                    do_chunks(m2, oh2, 1, first=(ND == 0), last=True)

                # ---- block epilogue ----
                aggTs = wpool.tile([D, BLK], f16, tag="aggTs")
                nc.scalar.activation(out=aggTs[:], in_=aggT[:],
                                     func=mybir.ActivationFunctionType.Copy)
                upd_ps = pupdpool.tile([128, 4 * D], f32, space="PSUM", tag="upd")
                for j in range(4):
                    nc.tensor.matmul(out=upd_ps[:, j * D:(j + 1) * D],
                                     lhsT=aggTs[:, j * 128:(j + 1) * 128],
                                     rhs=wupd_sb[:], start=True, stop=True)
                own_t = gpool.tile([128, 4 * D], f32, tag="own")
                nc.sync.dma_start(out=own_t[:], in_=own[b])
                o1 = opool.tile([128, 4 * D], f32, tag="o1")
                nc.vector.tensor_add(out=o1[:], in0=own_t[:], in1=upd_ps[:])
                ob = opool.tile([128, 4 * D], f32, tag="ob")
                nc.scalar.activation(out=ob[:], in_=o1[:],
                                     func=mybir.ActivationFunctionType.Relu)
                nc.sync.dma_start(out=out[b], in_=ob[:])

    nc.compile()
    return nc


# ------------------------------------------------------------------ helpers
def make_in_maps(inputs):
    """Host preprocessing -> (TB, list of per-core input dicts)."""
    pre, TB = host_preprocess(inputs["edge_rel"], inputs["edge_ab"],
                              inputs["edge_bc"], inputs["edge_ac"])
    tbl = build_table(inputs["edge_feat"], inputs["relation_emb"],
                      inputs["edge_rel"])
    iota = np.tile(np.arange(BLK, dtype=np.float16), (128, 1))
    wmsg = np.asarray(inputs["W_msg"], np.float32).astype(np.float16)
    wupd = np.asarray(inputs["W_upd"], np.float32).astype(np.float16)
    in_maps = []
    for k in range(NCORES):
        in_maps.append({
            "tbl": tbl,
            "own": build_own(tbl, k),
            "idx": pre[k]["idx"],
            "arh": pre[k]["arh"],
            "iota": iota,
            "wmsg": wmsg,
            "wupd": wupd,
        })
    return TB, in_maps


def run_full(inputs, nblk=NBLK):
    from concourse.bass_utils import run_bass_kernel_spmd
    import time as _time
    TB, in_maps = make_in_maps(inputs)
    t0 = _time.time()
    nc = build_bass(TB, nblk)
    print(f"[build+compile {_time.time()-t0:.1f}s TB={TB}]", flush=True)
    if nblk != NBLK:
        for m in in_maps:
            m["own"] = m["own"][:nblk]
            m["idx"] = m["idx"][:nblk]
            m["arh"] = m["arh"][:nblk]
    t0 = _time.time()
    res = run_bass_kernel_spmd(nc, in_maps, core_ids=list(range(NCORES)))
    print(f"[run1 {_time.time()-t0:.1f}s]", flush=True)
    if nblk != NBLK:
        return [res.results[k]["out"] for k in range(NCORES)]
    outs = [unpermute_out(res.results[k]["out"]) for k in range(NCORES)]
    return np.concatenate(outs, axis=0)


# ------------------------------------------------------------------ entry
def kernel(**inputs):
    """Self-contained entry: full unsharded inputs -> full [E, D] output."""
    out = run_full(inputs, nblk=NBLK)
    return np.asarray(out, np.float32)


# revision 10
# speedup vs baseline: 22.7576x; 22.7576x over previous
"""Trainium2 Bass kernel for the LogicMessagePassingNetwork problem.

Reference computation (E=1M edges, T=2M triangles, R=50, D=64):
    x   = edge_feat + relation_emb[edge_rel]                      # [E, D]
    m   = relu((x[edge_ab] * x[edge_bc]) @ W_msg)                 # [T, D]
    agg = segment_sum(m, edge_ac, E)                              # [E, D]
    out = relu(x + agg @ W_upd)                                   # [E, D]

Strategy (8 cores, no collectives):
  - Host folds relation_emb into the table: xtbl = edge_feat + rel_emb[rel]
    (one [E,64] f32 table; gathers on device fetch one row per triangle
    side instead of two).
  - Host sorts triangles by edge_ac; core k owns output edges
    [k*E/8, (k+1)*E/8) and exactly the triangles whose ac falls there.
  - Per core, output edges are processed in blocks of BLK=512. The
    block's triangles are padded to TB chunks of 128 slots (TB is the
    global max, ~9, so the program is identical across cores / SPMD).
  - Chunks are processed in dual-groups of 2: ONE indirect DMA gathers
    512 rows (xa,xb for both chunks) into a [128,256] tile; prod=xa*xb
    (DVE), PE-transpose, m=relu(prodT^T @ W_msg), one-hot [128,512]
    against the block's 512 edge slots (fp16), scatter-matmul
    accumulates aggT[64,512] f32 in PSUM.
  - Block epilogue: upd = aggT^T @ W_upd (4 matmuls), out =
    relu(own_x + upd) with own_x streamed as one contiguous 128KB DMA
    (host pre-permuted), written back as one 128KB DMA.
"""
import numpy as np

E = 1_000_000
T = 2_000_000
R = 50
D = 64
NCORES = 8
EPC = E // NCORES          # edges per core (125000)
BLK = 512                  # output edges per block
NBLK = (EPC + BLK - 1) // BLK          # 245 blocks/core
EPAD = NBLK * BLK                      # padded edges/core (125440)
TPAD = 512                             # extra zero rows at table end
ARPAD = 60000.0                        # fp16-exact, never matches iota


# ----------------------------------------------------------------- host prep
def host_preprocess(edge_rel, edge_ab, edge_bc, edge_ac):
    """Index-space preprocessing. Returns per-core index arrays + TB."""
    ab = np.asarray(edge_ab).astype(np.int64)
    bc = np.asarray(edge_bc).astype(np.int64)
    ac = np.asarray(edge_ac).astype(np.int64)

    order = np.argsort(ac, kind="stable")
    ab_s, bc_s, ac_s = ab[order], bc[order], ac[order]

    cores = []
    TB = 0
    for k in range(NCORES):
        lo, hi = np.searchsorted(ac_s, [k * EPC, (k + 1) * EPC])
        c_ab = ab_s[lo:hi].astype(np.int32)
        c_bc = bc_s[lo:hi].astype(np.int32)
        ac_l = ac_s[lo:hi] - k * EPC
        blk = ac_l // BLK
        cnt = np.bincount(blk, minlength=NBLK)
        TB = max(TB, -(-int(cnt.max()) // 128))
        cores.append((c_ab, c_bc, ac_l, blk, cnt))

    ND, NS = TB // 2, TB % 2
    outs = []
    for k in range(NCORES):
        c_ab, c_bc, ac_l, blk, cnt = cores[k]
        starts = np.zeros(NBLK, np.int64)
        starts[1:] = np.cumsum(cnt)[:-1]
        rank = np.arange(len(ac_l)) - starts[blk]
        chunk = rank // 128
        p = rank % 128

        gab = np.zeros((NBLK, TB, 128), np.int32)
        gbc = np.zeros((NBLK, TB, 128), np.int32)
        gar = np.full((NBLK, TB, 128), ARPAD, np.float16)
        gab[blk, chunk, p] = c_ab
        gbc[blk, chunk, p] = c_bc
        gar[blk, chunk, p] = (ac_l % BLK).astype(np.float16)

        idx = np.zeros((NBLK, 128, 2 * TB), np.int32)
        for g in range(ND):
            idx[:, :, 4 * g + 0] = gab[:, 2 * g, :]
            idx[:, :, 4 * g + 1] = gab[:, 2 * g + 1, :]
            idx[:, :, 4 * g + 2] = gbc[:, 2 * g, :]
            idx[:, :, 4 * g + 3] = gbc[:, 2 * g + 1, :]
        if NS:
            idx[:, :, 4 * ND + 0] = gab[:, TB - 1, :]
            idx[:, :, 4 * ND + 1] = gbc[:, TB - 1, :]
        arh = np.ascontiguousarray(gar.transpose(0, 2, 1))  # [NBLK,128,TB]
        outs.append(dict(idx=idx, arh=arh))
    return outs, TB


def build_table(edge_feat, relation_emb, edge_rel):
    """xtbl[i] = edge_feat[i] + relation_emb[edge_rel[i]], padded rows zero."""
    rel = np.asarray(edge_rel).astype(np.int64)
    tbl = np.zeros((E + TPAD, D), np.float32)
    tbl[:E] = np.asarray(edge_feat, np.float32) + \
        np.asarray(relation_emb, np.float32)[rel]
    return tbl


def build_own(tbl, k):
    """Pre-permuted own-edge x rows: own[b, p, j*64:(j+1)*64] = x[lo+b*512+j*128+p]."""
    ox = tbl[k * EPC: k * EPC + EPAD]                      # [125440, 64]
    return np.ascontiguousarray(
        ox.reshape(NBLK, 4, 128, D).transpose(0, 2, 1, 3).reshape(NBLK, 128, 4 * D))


def unpermute_out(o):
    """[NBLK,128,256] -> [EPC,64]"""
    return o.reshape(NBLK, 128, 4, D).transpose(0, 2, 1, 3).reshape(EPAD, D)[:EPC]


# ------------------------------------------------------------- device kernel
def build_bass(TB, nblk, split_gather=True, no_p64=True, no_f16=False):
    import concourse.bass as bass
    import concourse.bacc as bacc
    import concourse.mybir as mybir
    import concourse.tile as tile
    from concourse.masks import make_identity

    f32 = mybir.dt.float32
    f16 = mybir.dt.float32 if no_f16 else mybir.dt.float16
    i32 = mybir.dt.int32
    ND, NS = TB // 2, TB % 2
    nc = bacc.Bacc(None, target_bir_lowering=False)

    tbl = nc.dram_tensor("tbl", [E + TPAD, D], f32, kind="ExternalInput")
    own = nc.dram_tensor("own", [nblk, 128, 4 * D], f32, kind="ExternalInput")
    idx = nc.dram_tensor("idx", [nblk, 128, 2 * TB], i32, kind="ExternalInput")
    arh = nc.dram_tensor("arh", [nblk, 128, TB], f16, kind="ExternalInput")
    iota = nc.dram_tensor("iota", [128, BLK], f16, kind="ExternalInput")
    wmsg = nc.dram_tensor("wmsg", [D, D], f16, kind="ExternalInput")
    wupd = nc.dram_tensor("wupd", [D, D], f16, kind="ExternalInput")
    out = nc.dram_tensor("out", [nblk, 128, 4 * D], f32, kind="ExternalOutput")

    with tile.TileContext(nc) as tc:
        with tc.tile_pool(name="const", bufs=1) as cpool, \
             tc.tile_pool(name="gath", bufs=6) as gpool, \
             tc.tile_pool(name="idxp", bufs=3) as ipool, \
             tc.tile_pool(name="work", bufs=4) as wpool, \
             tc.tile_pool(name="ohp", bufs=4) as ohpool, \
             tc.tile_pool(name="outp", bufs=3) as opool, \
             tc.tile_pool(name="ps", bufs=2, space="PSUM") as pspool, \
             tc.tile_pool(name="psagg", bufs=2, space="PSUM") as paggpool, \
             tc.tile_pool(name="psupd", bufs=2, space="PSUM") as pupdpool:

            # W_msg duplicated on partitions 0:64 and 64:128 so matmuls with
            # lhsT at base partition 64 have rhs at the same base.
            wmsg_sb = cpool.tile([128, D], f16)
            nc.sync.dma_start(out=wmsg_sb[0:D, :], in_=wmsg[:])
            nc.sync.dma_start(out=wmsg_sb[D:128, :], in_=wmsg[:])
            wupd_sb = cpool.tile([D, D], f16)
            nc.sync.dma_start(out=wupd_sb[:], in_=wupd[:])
            iota_sb = cpool.tile([128, BLK], f16)
            nc.sync.dma_start(out=iota_sb[:], in_=iota[:])
            ident = cpool.tile([128, 128], f32)
            make_identity(nc, ident[:])

            for b in range(nblk):
                idx_t = ipool.tile([128, 2 * TB], i32, tag="idx")
                nc.sync.dma_start(out=idx_t[:], in_=idx[b])
                arh_t = ipool.tile([128, TB], f16, tag="arh")
                nc.sync.dma_start(out=arh_t[:], in_=arh[b])

                aggT = paggpool.tile([D, BLK], f32, space="PSUM", tag="aggT")

                def do_chunks(lhsT_m2, oh2, nch, first, last):
                    # scatter-accumulate nch chunks into aggT
                    for c in range(nch):
                        nc.tensor.matmul(
                            out=aggT[:],
                            lhsT=lhsT_m2[:, c * D:(c + 1) * D],
                            rhs=oh2[:, c * BLK:(c + 1) * BLK],
                            start=(first and c == 0), stop=(last and c == nch - 1))

                for g in range(ND):
                    gt = gpool.tile([128, 4 * D], f32, tag="g2")
                    if split_gather:
                        for j in range(4):
                            nc.gpsimd.indirect_dma_start(
                                out=gt[:, j * D:(j + 1) * D], out_offset=None,
                                in_=tbl[:],
                                in_offset=bass.IndirectOffsetOnAxis(
                                    ap=idx_t[:, 4 * g + j:4 * g + j + 1], axis=0))
                    else:
                        nc.gpsimd.indirect_dma_start(
                            out=gt[:], out_offset=None, in_=tbl[:],
                            in_offset=bass.IndirectOffsetOnAxis(
                                ap=idx_t[:, 4 * g:4 * g + 4], axis=0))
                    prod2 = wpool.tile([128, 128], f32, tag="prod2")
                    nc.vector.tensor_mul(out=prod2[:], in0=gt[:, 0:128],
                                         in1=gt[:, 128:256])
                    m_ps = pspool.tile([128, 128], f32, space="PSUM", tag="mps")
                    if no_p64:
                        # both chunks' prodT at base partition 0, side by side
                        prodTw = wpool.tile([D, 256], f16, tag="prodTw")
                        for h in range(2):
                            prodT_ps = pspool.tile([D, 128], f32, space="PSUM",
                                                   tag="prodT")
                            nc.tensor.transpose(out=prodT_ps[:],
                                                in_=prod2[:, h * D:(h + 1) * D],
                                                identity=ident[:])
                            nc.scalar.activation(
                                out=prodTw[:, h * 128:(h + 1) * 128],
                                in_=prodT_ps[:],
                                func=mybir.ActivationFunctionType.Copy)
                        for h in range(2):
                            nc.tensor.matmul(out=m_ps[:, h * D:(h + 1) * D],
                                             lhsT=prodTw[:, h * 128:(h + 1) * 128],
                                             rhs=wmsg_sb[0:D, :],
                                             start=True, stop=True)
                    else:
                        prodT = wpool.tile([128, 128], f16, tag="prodTs")
                        prodT_ps = pspool.tile([128, 128], f32, space="PSUM",
                                               tag="prodT")
                        nc.tensor.transpose(out=prodT_ps[:], in_=prod2[:],
                                            identity=ident[:])
                        nc.scalar.activation(out=prodT[:], in_=prodT_ps[:],
                                             func=mybir.ActivationFunctionType.Copy)
                        nc.tensor.matmul(out=m_ps[:, 0:D], lhsT=prodT[0:D, :],
                                         rhs=wmsg_sb[0:D, :], start=True, stop=True)
                        nc.tensor.matmul(out=m_ps[:, D:2 * D], lhsT=prodT[D:128, :],
                                         rhs=wmsg_sb[D:128, :], start=True, stop=True)
                    m2 = wpool.tile([128, 128], f16, tag="m2")
                    nc.scalar.activation(out=m2[:], in_=m_ps[:],
                                         func=mybir.ActivationFunctionType.Relu)
                    oh2 = ohpool.tile([128, 2 * BLK], f16, tag="oh2")
                    nc.vector.tensor_tensor(
                        out=oh2[:, 0:BLK],
                        in0=arh_t[:, 2 * g:2 * g + 1].to_broadcast([128, BLK]),
                        in1=iota_sb[:], op=mybir.AluOpType.is_equal)
                    nc.vector.tensor_tensor(
                        out=oh2[:, BLK:2 * BLK],
                        in0=arh_t[:, 2 * g + 1:2 * g + 2].to_broadcast([128, BLK]),
                        in1=iota_sb[:], op=mybir.AluOpType.is_equal)
                    do_chunks(m2, oh2, 2, first=(g == 0), last=(NS == 0 and g == ND - 1))

                if NS:
                    gt = gpool.tile([128, 2 * D], f32, tag="g1")
                    if split_gather:
                        for j in range(2):
                            nc.gpsimd.indirect_dma_start(
                                out=gt[:, j * D:(j + 1) * D], out_offset=None,
                                in_=tbl[:],
                                in_offset=bass.IndirectOffsetOnAxis(
                                    ap=idx_t[:, 4 * ND + j:4 * ND + j + 1], axis=0))
                    else:
                        nc.gpsimd.indirect_dma_start(
                            out=gt[:], out_offset=None, in_=tbl[:],
                            in_offset=bass.IndirectOffsetOnAxis(
                                ap=idx_t[:, 4 * ND:4 * ND + 2], axis=0))
                    prod1 = wpool.tile([128, D], f32, tag="prod1")
                    nc.vector.tensor_mul(out=prod1[:], in0=gt[:, 0:D],
                                         in1=gt[:, D:2 * D])
                    prodT_ps = pspool.tile([128, 128], f32, space="PSUM", tag="prodT")
                    nc.tensor.transpose(out=prodT_ps[0:D, :], in_=prod1[:],
                                        identity=ident[:])
                    prodT = wpool.tile([128, 128], f16, tag="prodTs")
                    nc.scalar.activation(out=prodT[0:D, :], in_=prodT_ps[0:D, :],
                                         func=mybir.ActivationFunctionType.Copy)
                    m_ps = pspool.tile([128, 128], f32, space="PSUM", tag="mps")
                    nc.tensor.matmul(out=m_ps[:, 0:D], lhsT=prodT[0:D, :],
                                     rhs=wmsg_sb[0:D, :], start=True, stop=True)
                    m2 = wpool.tile([128, 128], f16, tag="m2")
                    nc.scalar.activation(out=m2[:, 0:D], in_=m_ps[:, 0:D],
                                         func=mybir.ActivationFunctionType.Relu)
                    oh2 = ohpool.tile([128, 2 * BLK], f16, tag="oh2")
                    nc.vector.tensor_tensor(
                        out=oh2[:, 0:BLK],
                        in0=arh_t[:, TB - 1:TB].to_broadcast([128, BLK]),
                        in1=iota_sb[:], op=mybir.AluOpType.is_equal)
                    do_chunks(m2, oh2, 1, first=(ND == 0), last=True)

                # ---- block epilogue ----
                aggTs = wpool.tile([D, BLK], f16, tag="aggTs")
                nc.scalar.activation(out=aggTs[:], in_=aggT[:],
                                     func=mybir.ActivationFunctionType.Copy)
                upd_ps = pupdpool.tile([128, 4 * D], f32, space="PSUM", tag="upd")
                for j in range(4):
                    nc.tensor.matmul(out=upd_ps[:, j * D:(j + 1) * D],
                                     lhsT=aggTs[:, j * 128:(j + 1) * 128],
                                     rhs=wupd_sb[:], start=True, stop=True)
                own_t = gpool.tile([128, 4 * D], f32, tag="own")
                nc.sync.dma_start(out=own_t[:], in_=own[b])
                o1 = opool.tile([128, 4 * D], f32, tag="o1")
                nc.vector.tensor_add(out=o1[:], in0=own_t[:], in1=upd_ps[:])
                ob = opool.tile([128, 4 * D], f32, tag="ob")
                nc.scalar.activation(out=ob[:], in_=o1[:],
                                     func=mybir.ActivationFunctionType.Relu)
                nc.sync.dma_start(out=out[b], in_=ob[:])

    nc.compile()
    return nc


# ------------------------------------------------------------------ helpers
def make_in_maps(inputs):
    """Host preprocessing -> (TB, list of per-core input dicts)."""
    pre, TB = host_preprocess(inputs["edge_rel"], inputs["edge_ab"],
                              inputs["edge_bc"], inputs["edge_ac"])
    tbl = build_table(inputs["edge_feat"], inputs["relation_emb"],
                      inputs["edge_rel"])
    iota = np.tile(np.arange(BLK, dtype=np.float16), (128, 1))
    wmsg = np.asarray(inputs["W_msg"], np.float32).astype(np.float16)
    wupd = np.asarray(inputs["W_upd"], np.float32).astype(np.float16)
    in_maps = []
    for k in range(NCORES):
        in_maps.append({
            "tbl": tbl,
            "own": build_own(tbl, k),
            "idx": pre[k]["idx"],
            "arh": pre[k]["arh"],
            "iota": iota,
            "wmsg": wmsg,
            "wupd": wupd,
        })
    return TB, in_maps


def run_full(inputs, nblk=NBLK):
    from concourse.bass_utils import run_bass_kernel_spmd
    import time as _time
    TB, in_maps = make_in_maps(inputs)
    t0 = _time.time()
    nc = build_bass(TB, nblk)
    print(f"[build+compile {_time.time()-t0:.1f}s TB={TB}]", flush=True)
    if nblk != NBLK:
        for m in in_maps:
            m["own"] = m["own"][:nblk]
            m["idx"] = m["idx"][:nblk]
            m["arh"] = m["arh"][:nblk]
    t0 = _time.time()
    res = run_bass_kernel_spmd(nc, in_maps, core_ids=list(range(NCORES)))
    print(f"[run1 {_time.time()-t0:.1f}s]", flush=True)
    if nblk != NBLK:
        return [res.results[k]["out"] for k in range(NCORES)]
    outs = [unpermute_out(res.results[k]["out"]) for k in range(NCORES)]
    return np.concatenate(outs, axis=0)


# ------------------------------------------------------------------ entry
def kernel(**inputs):
    """Self-contained entry: full unsharded inputs -> full [E, D] output."""
    out = run_full(inputs, nblk=NBLK)
    return np.asarray(out, np.float32)


# revision 14
# speedup vs baseline: 22.9469x; 1.0083x over previous
"""Trainium2 Bass kernel for the LogicMessagePassingNetwork problem.

Reference computation (E=1M edges, T=2M triangles, R=50, D=64):
    x   = edge_feat + relation_emb[edge_rel]                      # [E, D]
    m   = relu((x[edge_ab] * x[edge_bc]) @ W_msg)                 # [T, D]
    agg = segment_sum(m, edge_ac, E)                              # [E, D]
    out = relu(x + agg @ W_upd)                                   # [E, D]

Strategy (8 cores, no collectives):
  - Host folds relation_emb into the table: xtbl = edge_feat + rel_emb[rel]
    (one [E,64] f32 table; gathers on device fetch one row per triangle
    side instead of two).
  - Host sorts triangles by edge_ac; core k owns output edges
    [k*E/8, (k+1)*E/8) and exactly the triangles whose ac falls there.
  - Per core, output edges are processed in blocks of BLK=512. The
    block's triangles are padded to TB chunks of 128 slots (TB is the
    global max, ~9, so the program is identical across cores / SPMD).
  - Chunks are processed in dual-groups of 2: ONE indirect DMA gathers
    512 rows (xa,xb for both chunks) into a [128,256] tile; prod=xa*xb
    (DVE), PE-transpose, m=relu(prodT^T @ W_msg), one-hot [128,512]
    against the block's 512 edge slots (fp16), scatter-matmul
    accumulates aggT[64,512] f32 in PSUM.
  - Block epilogue: upd = aggT^T @ W_upd (4 matmuls), out =
    relu(own_x + upd) with own_x streamed as one contiguous 128KB DMA
    (host pre-permuted), written back as one 128KB DMA.
"""
import numpy as np

E = 1_000_000
T = 2_000_000
R = 50
D = 64
NCORES = 8
EPC = E // NCORES          # edges per core (125000)
BLK = 512                  # output edges per block
NBLK = (EPC + BLK - 1) // BLK          # 245 blocks/core
EPAD = NBLK * BLK                      # padded edges/core (125440)
TPAD = 512                             # extra zero rows at table end
ARPAD = 60000.0                        # fp16-exact, never matches iota
PADIDX = 100_000_000                   # OOB: bounds_check skips these rows


# ----------------------------------------------------------------- host prep
def host_preprocess(edge_rel, edge_ab, edge_bc, edge_ac):
    """Index-space preprocessing. Returns per-core index arrays + TB."""
    ab = np.asarray(edge_ab).astype(np.int64)
    bc = np.asarray(edge_bc).astype(np.int64)
    ac = np.asarray(edge_ac).astype(np.int64)

    order = np.argsort(ac, kind="stable")
    ab_s, bc_s, ac_s = ab[order], bc[order], ac[order]

    cores = []
    TB = 0
    for k in range(NCORES):
        lo, hi = np.searchsorted(ac_s, [k * EPC, (k + 1) * EPC])
        c_ab = ab_s[lo:hi].astype(np.int32)
        c_bc = bc_s[lo:hi].astype(np.int32)
        ac_l = ac_s[lo:hi] - k * EPC
        blk = ac_l // BLK
        cnt = np.bincount(blk, minlength=NBLK)
        TB = max(TB, -(-int(cnt.max()) // 128))
        cores.append((c_ab, c_bc, ac_l, blk, cnt))

    ND, NS = TB // 2, TB % 2
    outs = []
    for k in range(NCORES):
        c_ab, c_bc, ac_l, blk, cnt = cores[k]
        starts = np.zeros(NBLK, np.int64)
        starts[1:] = np.cumsum(cnt)[:-1]
        rank = np.arange(len(ac_l)) - starts[blk]
        chunk = rank // 128
        p = rank % 128

        gab = np.full((NBLK, TB, 128), PADIDX, np.int32)
        gbc = np.full((NBLK, TB, 128), PADIDX, np.int32)
        gar = np.full((NBLK, TB, 128), ARPAD, np.float16)
        gab[blk, chunk, p] = c_ab
        gbc[blk, chunk, p] = c_bc
        gar[blk, chunk, p] = (ac_l % BLK).astype(np.float16)

        idx = np.zeros((NBLK, 128, 2 * TB), np.int32)
        for g in range(ND):
            idx[:, :, 4 * g + 0] = gab[:, 2 * g, :]
            idx[:, :, 4 * g + 1] = gab[:, 2 * g + 1, :]
            idx[:, :, 4 * g + 2] = gbc[:, 2 * g, :]
            idx[:, :, 4 * g + 3] = gbc[:, 2 * g + 1, :]
        if NS:
            idx[:, :, 4 * ND + 0] = gab[:, TB - 1, :]
            idx[:, :, 4 * ND + 1] = gbc[:, TB - 1, :]
        arh = np.ascontiguousarray(gar.transpose(0, 2, 1))  # [NBLK,128,TB]
        outs.append(dict(idx=idx, arh=arh))
    return outs, TB


def build_table(edge_feat, relation_emb, edge_rel):
    """xtbl[i] = edge_feat[i] + relation_emb[edge_rel[i]], padded rows zero."""
    rel = np.asarray(edge_rel).astype(np.int64)
    tbl = np.zeros((E + TPAD, D), np.float32)
    tbl[:E] = np.asarray(edge_feat, np.float32) + \
        np.asarray(relation_emb, np.float32)[rel]
    return tbl


def build_own(tbl, k):
    """Pre-permuted own-edge x rows: own[b, p, j*64:(j+1)*64] = x[lo+b*512+j*128+p]."""
    ox = tbl[k * EPC: k * EPC + EPAD]                      # [125440, 64]
    return np.ascontiguousarray(
        ox.reshape(NBLK, 4, 128, D).transpose(0, 2, 1, 3).reshape(NBLK, 128, 4 * D))


def unpermute_out(o):
    """[NBLK,128,256] -> [EPC,64]"""
    return o.reshape(NBLK, 128, 4, D).transpose(0, 2, 1, 3).reshape(EPAD, D)[:EPC]


# ------------------------------------------------------------- device kernel
def build_bass(TB, nblk, split_gather=True, no_p64=True, no_f16=False,
               no_gather=False, only_gather=False, half_gather=False,
               gbufs=6, wbufs=4, obufs=4):
    import concourse.bass as bass
    import concourse.bacc as bacc
    import concourse.mybir as mybir
    import concourse.tile as tile
    from concourse.masks import make_identity

    f32 = mybir.dt.float32
    f16 = mybir.dt.float32 if no_f16 else mybir.dt.float16
    i32 = mybir.dt.int32
    ND, NS = TB // 2, TB % 2
    nc = bacc.Bacc(None, target_bir_lowering=False)

    tbl = nc.dram_tensor("tbl", [E + TPAD, D], f32, kind="ExternalInput")
    own = nc.dram_tensor("own", [nblk, 128, 4 * D], f32, kind="ExternalInput")
    idx = nc.dram_tensor("idx", [nblk, 128, 2 * TB], i32, kind="ExternalInput")
    arh = nc.dram_tensor("arh", [nblk, 128, TB], f16, kind="ExternalInput")
    iota = nc.dram_tensor("iota", [128, BLK], f16, kind="ExternalInput")
    wmsg = nc.dram_tensor("wmsg", [D, D], f16, kind="ExternalInput")
    wupd = nc.dram_tensor("wupd", [D, D], f16, kind="ExternalInput")
    out = nc.dram_tensor("out", [nblk, 128, 4 * D], f32, kind="ExternalOutput")

    with tile.TileContext(nc) as tc:
        with tc.tile_pool(name="const", bufs=1) as cpool, \
             tc.tile_pool(name="gath", bufs=gbufs) as gpool, \
             tc.tile_pool(name="idxp", bufs=3) as ipool, \
             tc.tile_pool(name="work", bufs=wbufs) as wpool, \
             tc.tile_pool(name="ohp", bufs=obufs) as ohpool, \
             tc.tile_pool(name="outp", bufs=3) as opool, \
             tc.tile_pool(name="ps", bufs=2, space="PSUM") as pspool, \
             tc.tile_pool(name="psagg", bufs=2, space="PSUM") as paggpool, \
             tc.tile_pool(name="psupd", bufs=2, space="PSUM") as pupdpool:

            # W_msg duplicated on partitions 0:64 and 64:128 so matmuls with
            # lhsT at base partition 64 have rhs at the same base.
            wmsg_sb = cpool.tile([128, D], f16)
            nc.sync.dma_start(out=wmsg_sb[0:D, :], in_=wmsg[:])
            nc.sync.dma_start(out=wmsg_sb[D:128, :], in_=wmsg[:])
            wupd_sb = cpool.tile([D, D], f16)
            nc.sync.dma_start(out=wupd_sb[:], in_=wupd[:])
            iota_sb = cpool.tile([128, BLK], f16)
            nc.sync.dma_start(out=iota_sb[:], in_=iota[:])
            ident = cpool.tile([128, 128], f32)
            make_identity(nc, ident[:])

            for b in range(nblk):
                idx_t = ipool.tile([128, 2 * TB], i32, tag="idx")
                nc.sync.dma_start(out=idx_t[:], in_=idx[b])
                arh_t = ipool.tile([128, TB], f16, tag="arh")
                nc.sync.dma_start(out=arh_t[:], in_=arh[b])

                aggT = paggpool.tile([D, BLK], f32, space="PSUM", tag="aggT")

                def do_chunks(lhsT_m2, oh2, nch, first, last):
                    # scatter-accumulate nch chunks into aggT
                    for c in range(nch):
                        nc.tensor.matmul(
                            out=aggT[:],
                            lhsT=lhsT_m2[:, c * D:(c + 1) * D],
                            rhs=oh2[:, c * BLK:(c + 1) * BLK],
                            start=(first and c == 0), stop=(last and c == nch - 1))

                for g in range(ND):
                    gt = gpool.tile([128, 4 * D], f32, tag="g2")
                    if no_gather:
                        pass
                    elif split_gather:
                        nj = 2 if half_gather else 4
                        for j in range(nj):
                            nc.gpsimd.indirect_dma_start(
                                out=gt[:, j * D:(j + 1) * D], out_offset=None,
                                in_=tbl[:],
                                in_offset=bass.IndirectOffsetOnAxis(
                                    ap=idx_t[:, 4 * g + j:4 * g + j + 1], axis=0),
                                bounds_check=E + TPAD - 1, oob_is_err=False)
                    else:
                        nc.gpsimd.indirect_dma_start(
                            out=gt[:], out_offset=None, in_=tbl[:],
                            in_offset=bass.IndirectOffsetOnAxis(
                                ap=idx_t[:, 4 * g:4 * g + 4], axis=0))
                    if only_gather:
                        continue
                    prod2 = wpool.tile([128, 128], f32, tag="prod2")
                    nc.vector.tensor_mul(out=prod2[:], in0=gt[:, 0:128],
                                         in1=gt[:, 128:256])
                    m_ps = pspool.tile([128, 128], f32, space="PSUM", tag="mps")
                    if no_p64:
                        # both chunks' prodT at base partition 0, side by side
                        prodTw = wpool.tile([D, 256], f16, tag="prodTw")
                        for h in range(2):
                            prodT_ps = pspool.tile([D, 128], f32, space="PSUM",
                                                   tag="prodT")
                            nc.tensor.transpose(out=prodT_ps[:],
                                                in_=prod2[:, h * D:(h + 1) * D],
                                                identity=ident[:])
                            nc.scalar.activation(
                                out=prodTw[:, h * 128:(h + 1) * 128],
                                in_=prodT_ps[:],
                                func=mybir.ActivationFunctionType.Copy)
                        for h in range(2):
                            nc.tensor.matmul(out=m_ps[:, h * D:(h + 1) * D],
                                             lhsT=prodTw[:, h * 128:(h + 1) * 128],
                                             rhs=wmsg_sb[0:D, :],
                                             start=True, stop=True)
                    else:
                        prodT = wpool.tile([128, 128], f16, tag="prodTs")
                        prodT_ps = pspool.tile([128, 128], f32, space="PSUM",
                                               tag="prodT")
                        nc.tensor.transpose(out=prodT_ps[:], in_=prod2[:],
                                            identity=ident[:])
                        nc.scalar.activation(out=prodT[:], in_=prodT_ps[:],
                                             func=mybir.ActivationFunctionType.Copy)
                        nc.tensor.matmul(out=m_ps[:, 0:D], lhsT=prodT[0:D, :],
                                         rhs=wmsg_sb[0:D, :], start=True, stop=True)
                        nc.tensor.matmul(out=m_ps[:, D:2 * D], lhsT=prodT[D:128, :],
                                         rhs=wmsg_sb[D:128, :], start=True, stop=True)
                    m2 = wpool.tile([128, 128], f16, tag="m2")
                    nc.scalar.activation(out=m2[:], in_=m_ps[:],
                                         func=mybir.ActivationFunctionType.Relu)
                    oh2 = ohpool.tile([128, 2 * BLK], f16, tag="oh2")
                    nc.vector.tensor_tensor(
                        out=oh2[:, 0:BLK],
                        in0=arh_t[:, 2 * g:2 * g + 1].to_broadcast([128, BLK]),
                        in1=iota_sb[:], op=mybir.AluOpType.is_equal)
                    nc.vector.tensor_tensor(
                        out=oh2[:, BLK:2 * BLK],
                        in0=arh_t[:, 2 * g + 1:2 * g + 2].to_broadcast([128, BLK]),
                        in1=iota_sb[:], op=mybir.AluOpType.is_equal)
                    do_chunks(m2, oh2, 2, first=(g == 0), last=(NS == 0 and g == ND - 1))

                if NS:
                    gt = gpool.tile([128, 2 * D], f32, tag="g1")
                    if no_gather:
                        pass
                    elif split_gather:
                        nj = 1 if half_gather else 2
                        for j in range(nj):
                            nc.gpsimd.indirect_dma_start(
                                out=gt[:, j * D:(j + 1) * D], out_offset=None,
                                in_=tbl[:],
                                in_offset=bass.IndirectOffsetOnAxis(
                                    ap=idx_t[:, 4 * ND + j:4 * ND + j + 1], axis=0),
                                bounds_check=E + TPAD - 1, oob_is_err=False)
                    else:
                        nc.gpsimd.indirect_dma_start(
                            out=gt[:], out_offset=None, in_=tbl[:],
                            in_offset=bass.IndirectOffsetOnAxis(
                                ap=idx_t[:, 4 * ND:4 * ND + 2], axis=0))
                    prod1 = wpool.tile([128, D], f32, tag="prod1")
                    if not only_gather:
                        nc.vector.tensor_mul(out=prod1[:], in0=gt[:, 0:D],
                                             in1=gt[:, D:2 * D])
                    prodT_ps = pspool.tile([128, 128], f32, space="PSUM", tag="prodT")
                    if not only_gather:
                        nc.tensor.transpose(out=prodT_ps[0:D, :], in_=prod1[:],
                                            identity=ident[:])
                    if not only_gather:
                        prodT = wpool.tile([128, 128], f16, tag="prodTs")
                        nc.scalar.activation(out=prodT[0:D, :], in_=prodT_ps[0:D, :],
                                             func=mybir.ActivationFunctionType.Copy)
                        m_ps = pspool.tile([128, 128], f32, space="PSUM", tag="mps")
                        nc.tensor.matmul(out=m_ps[:, 0:D], lhsT=prodT[0:D, :],
                                         rhs=wmsg_sb[0:D, :], start=True, stop=True)
                        m2 = wpool.tile([128, 128], f16, tag="m2")
                        nc.scalar.activation(out=m2[:, 0:D], in_=m_ps[:, 0:D],
                                             func=mybir.ActivationFunctionType.Relu)
                        oh2 = ohpool.tile([128, 2 * BLK], f16, tag="oh2")
                        nc.vector.tensor_tensor(
                            out=oh2[:, 0:BLK],
                            in0=arh_t[:, TB - 1:TB].to_broadcast([128, BLK]),
                            in1=iota_sb[:], op=mybir.AluOpType.is_equal)
                        do_chunks(m2, oh2, 1, first=(ND == 0), last=True)

                # ---- block epilogue ----
                own_t = gpool.tile([128, 4 * D], f32, tag="own")
                nc.sync.dma_start(out=own_t[:], in_=own[b])
                if only_gather:
                    nc.sync.dma_start(out=out[b], in_=own_t[:])
                    continue
                aggTs = wpool.tile([D, BLK], f16, tag="aggTs")
                nc.scalar.activation(out=aggTs[:], in_=aggT[:],
                                     func=mybir.ActivationFunctionType.Copy)
                upd_ps = pupdpool.tile([128, 4 * D], f32, space="PSUM", tag="upd")
                for j in range(4):
                    nc.tensor.matmul(out=upd_ps[:, j * D:(j + 1) * D],
                                     lhsT=aggTs[:, j * 128:(j + 1) * 128],
                                     rhs=wupd_sb[:], start=True, stop=True)
                o1 = opool.tile([128, 4 * D], f32, tag="o1")
                nc.vector.tensor_add(out=o1[:], in0=own_t[:], in1=upd_ps[:])
                ob = opool.tile([128, 4 * D], f32, tag="ob")
                nc.scalar.activation(out=ob[:], in_=o1[:],
                                     func=mybir.ActivationFunctionType.Relu)
                nc.sync.dma_start(out=out[b], in_=ob[:])

    nc.compile()
    return nc


# ------------------------------------------------------------------ helpers
def make_in_maps(inputs):
    """Host preprocessing -> (TB, list of per-core input dicts)."""
    pre, TB = host_preprocess(inputs["edge_rel"], inputs["edge_ab"],
                              inputs["edge_bc"], inputs["edge_ac"])
    tbl = build_table(inputs["edge_feat"], inputs["relation_emb"],
                      inputs["edge_rel"])
    iota = np.tile(np.arange(BLK, dtype=np.float16), (128, 1))
    wmsg = np.asarray(inputs["W_msg"], np.float32).astype(np.float16)
    wupd = np.asarray(inputs["W_upd"], np.float32).astype(np.float16)
    in_maps = []
    for k in range(NCORES):
        in_maps.append({
            "tbl": tbl,
            "own": build_own(tbl, k),
            "idx": pre[k]["idx"],
            "arh": pre[k]["arh"],
            "iota": iota,
            "wmsg": wmsg,
            "wupd": wupd,
        })
    return TB, in_maps


def run_full(inputs, nblk=NBLK):
    from concourse.bass_utils import run_bass_kernel_spmd
    import time as _time
    TB, in_maps = make_in_maps(inputs)
    t0 = _time.time()
    nc = build_bass(TB, nblk)
    print(f"[build+compile {_time.time()-t0:.1f}s TB={TB}]", flush=True)
    if nblk != NBLK:
        for m in in_maps:
            m["own"] = m["own"][:nblk]
            m["idx"] = m["idx"][:nblk]
            m["arh"] = m["arh"][:nblk]
    t0 = _time.time()
    res = run_bass_kernel_spmd(nc, in_maps, core_ids=list(range(NCORES)))
    print(f"[run1 {_time.time()-t0:.1f}s]", flush=True)
    if nblk != NBLK:
        return [res.results[k]["out"] for k in range(NCORES)]
    outs = [unpermute_out(res.results[k]["out"]) for k in range(NCORES)]
    return np.concatenate(outs, axis=0)


# ------------------------------------------------------------------ entry
def kernel(**inputs):
    """Self-contained entry: full unsharded inputs -> full [E, D] output."""
    out = run_full(inputs, nblk=NBLK)
    return np.asarray(out, np.float32)
